# revision 3
# baseline (speedup 1.0000x reference)
"""Bass/Tile kernel for nn_BidirectionalAttention on 8 trn2 NeuronCores.

Sharding: data-parallel over batch (4) x tensor-parallel over head groups (2).
Core c = 2*b + g handles batch b, heads [8g..8g+8) (columns [512g..512(g+1))
of wq/wk/wv, rows of wo). Each core produces a partial output projection;
the host sums the two head-group partials per batch and adds bo.

Device dataflow (per core, all matmuls in f32r):
  xT [c,s] (host-transposed)  --PE-->  qT,kT [d,s] (+bias via DVE evac)
  xT, wv                      --PE-->  v [s,d] (+bias, x key-mask, +ones col)
  S^T[k,q] = kT_h^T q (head pairs run concurrently in the PE array via
  row-halves)              --ACT-->  probs = exp(S^T/8) (f32r)
  attnT_aug[d+1,q] = [v_h|m01]^T probs (rowsum rides as the 65th row;
  masked keys contribute 0 to both numerator and denominator)
  attnT = attnT_aug[:64] * recip(rowsum)  --PE row-pairs--> out partial.
"""

import sys

sys.path.insert(0, "/opt/trn_rl_repo")
import numpy as np

B, S, H = 4, 1024, 1024
NH, HD = 16, 64
NCORES, GROUPS = 8, 2
DSH = H // GROUPS  # 512 shard width
NHL = NH // GROUPS  # 8 local heads
CT = H // 128  # 8 contraction tiles
ST = S // 128  # 8 s/k tiles
MT = DSH // 128  # 4 d' tiles (= head pairs)
QBS = 512
QB = S // QBS  # 2 q blocks
KTC = ST // 2  # 4 k-chunks of 2 tiles

_cache: dict = {}


def _body(tc, nc, mybir, d):
    import concourse.bass as bass

    F32 = mybir.dt.float32
    F32R = mybir.dt.float32r
    EXP = mybir.ActivationFunctionType.Exp
    from contextlib import ExitStack

    with ExitStack() as ctx:
        persist = ctx.enter_context(tc.tile_pool(name="persist", bufs=1))
        xtp = ctx.enter_context(tc.tile_pool(name="xtp", bufs=1))
        evac = ctx.enter_context(tc.tile_pool(name="evac", bufs=2))
        smalls = ctx.enter_context(tc.tile_pool(name="smalls", bufs=2))
        mmps = ctx.enter_context(tc.tile_pool(name="mmps", bufs=2, space="PSUM"))
        scps = ctx.enter_context(tc.tile_pool(name="scps", bufs=1, space="PSUM"))
        pvps = ctx.enter_context(tc.tile_pool(name="pvps", bufs=1, space="PSUM"))

        wq_sb = persist.tile([128, CT, DSH], F32R)
        wk_sb = persist.tile([128, CT, DSH], F32R)
        wo_sb = persist.tile([128, MT, H], F32R)
        qt_sb = persist.tile([128, MT, S], F32R)
        kt_sb = persist.tile([128, MT, S], F32R)
        at_sb = persist.tile([128, MT, S], F32R)
        v_sb = persist.tile([128, ST, NHL, HD + 1], F32R)
        bq_sb = persist.tile([128, MT], F32)
        bk_sb = persist.tile([128, MT], F32)
        bvb_sb = persist.tile([128, DSH], F32)
        m01_sb = persist.tile([128, ST], F32)
        xt_sb = xtp.tile([128, CT, S], F32R)

        with tc.tile_pool(name="wvp", bufs=1) as wvp:
            wv_sb = wvp.tile([128, CT, DSH], F32R)
            # -------- loads (v/x first: v-projection is the first consumer)
            for j in range(CT):
                nc.sync.dma_start(wv_sb[:, j, :], d["wv"][j * 128 : (j + 1) * 128, :])
                nc.sync.dma_start(xt_sb[:, j, :], d["xt"][j * 128 : (j + 1) * 128, :])
            nc.sync.dma_start(bvb_sb[:], d["bvb"])
            nc.sync.dma_start(m01_sb[:], d["m01"])
            nc.sync.dma_start(v_sb[:, :, :, HD : HD + 1], d["m01c"])
            nc.sync.dma_start(bq_sb[:], d["bq"])
            nc.sync.dma_start(bk_sb[:], d["bk"])
            for j in range(CT):
                nc.sync.dma_start(wq_sb[:, j, :], d["wq"][j * 128 : (j + 1) * 128, :])
                nc.sync.dma_start(wk_sb[:, j, :], d["wk"][j * 128 : (j + 1) * 128, :])
            nc.sync.dma_start(wo_sb[:], d["wo"].rearrange("(m p) h -> p m h", p=128))

            # -------- v projection: v[s,d'] = x @ wv + bv, then key-mask
            for m in range(ST):
                ps = mmps.tile([128, QBS], F32, tag="ps")
                for j in range(CT):
                    nc.tensor.matmul(
                        ps[:],
                        xt_sb[:, j, m * 128 : (m + 1) * 128],
                        wv_sb[:, j, :],
                        start=(j == 0),
                        stop=(j == CT - 1),
                    )
                tmp = evac.tile([128, DSH], F32, tag="tmp")
                nc.vector.tensor_add(tmp[:], ps[:], bvb_sb[:])
                nc.vector.tensor_scalar_mul(
                    v_sb[:, m, :, 0:HD],
                    tmp[:].rearrange("p (h e) -> p h e", h=NHL),
                    m01_sb[:, m : m + 1],
                )

        with tc.tile_pool(name="probs", bufs=2) as probs:
            # -------- per head-pair: q/k projections then attention
            for m in range(MT):
                for w_sb, b_sb, o_sb in (
                    (wq_sb, bq_sb, qt_sb),
                    (wk_sb, bk_sb, kt_sb),
                ):
                    for n in range(QB):
                        ps = mmps.tile([128, QBS], F32, tag="ps")
                        for j in range(CT):
                            nc.tensor.matmul(
                                ps[:],
                                w_sb[:, j, m * 128 : (m + 1) * 128],
                                xt_sb[:, j, n * QBS : (n + 1) * QBS],
                                start=(j == 0),
                                stop=(j == CT - 1),
                            )
                        nc.vector.tensor_scalar_add(
                            o_sb[:, m, n * QBS : (n + 1) * QBS],
                            ps[:],
                            b_sb[:, m : m + 1],
                        )
                for qb in range(QB):
                    pvA = pvps.tile([HD + 1, QBS], F32, tag="pvA")
                    pvB = pvps.tile([HD + 1, QBS], F32, tag="pvB")
                    for c in range(KTC):
                        scA = scps.tile([128, 2 * QBS], F32, tag="scA")
                        scB = scps.tile([128, 2 * QBS], F32, tag="scB")
                        for half in range(2):
                            kt_i = 2 * c + half
                            for sc, hb in ((scA, 0), (scB, 64)):
                                nc.tensor.matmul(
                                    sc[:, half * QBS : (half + 1) * QBS],
                                    kt_sb[hb : hb + 64, m, kt_i * 128 : (kt_i + 1) * 128],
                                    qt_sb[hb : hb + 64, m, qb * QBS : (qb + 1) * QBS],
                                    start=True,
                                    stop=True,
                                )
                        prA = probs.tile([128, 2, QBS], F32R, tag="prA")
                        prB = probs.tile([128, 2, QBS], F32R, tag="prB")
                        nc.scalar.activation(
                            prA[:].rearrange("p a b -> p (a b)"), scA[:], EXP, scale=0.125
                        )
                        nc.scalar.activation(
                            prB[:].rearrange("p a b -> p (a b)"), scB[:], EXP, scale=0.125
                        )
                        for half in range(2):
                            kt_i = 2 * c + half
                            for pv, pr, h in ((pvA, prA, 2 * m), (pvB, prB, 2 * m + 1)):
                                nc.tensor.matmul(
                                    pv[:],
                                    v_sb[:, kt_i, h, :],
                                    pr[:, half, :],
                                    start=(kt_i == 0),
                                    stop=(kt_i == ST - 1),
                                )
                    for pv, hb in ((pvA, 0), (pvB, 64)):
                        r = smalls.tile([1, QBS], F32, tag="r")
                        nc.vector.reciprocal(r[:], pv[HD : HD + 1, :])
                        rb = smalls.tile([64, QBS], F32, tag="rb")
                        nc.gpsimd.partition_broadcast(rb[:], r[:])
                        nc.vector.tensor_mul(
                            at_sb[hb : hb + 64, m, qb * QBS : (qb + 1) * QBS],
                            pv[0:HD, :],
                            rb[:],
                        )

            # -------- output projection (partial: host sums head groups + bo)
            for mq in range(ST):
                for n in range(QB):
                    ps = mmps.tile([128, QBS], F32, tag="ps")
                    for j in range(MT):
                        nc.tensor.matmul(
                            ps[:],
                            at_sb[:, j, mq * 128 : (mq + 1) * 128],
                            wo_sb[:, j, n * QBS : (n + 1) * QBS],
                            start=(j == 0),
                            stop=(j == MT - 1),
                        )
                    o = evac.tile([128, QBS], F32, tag="o")
                    nc.vector.tensor_copy(o[:], ps[:])
                    nc.sync.dma_start(
                        d["outp"][mq * 128 : (mq + 1) * 128, n * QBS : (n + 1) * QBS],
                        o[:],
                    )


def _build():
    if "nc" in _cache:
        return _cache["nc"]
    import concourse.tile as tile
    from concourse import bacc, mybir

    F32 = mybir.dt.float32
    F32R = mybir.dt.float32r
    nc = bacc.Bacc(
        "TRN2", target_bir_lowering=False, debug=False, num_devices=NCORES
    )
    d = {
        "xt": nc.dram_tensor("xt", [H, S], F32R, kind="ExternalInput").ap(),
        "wq": nc.dram_tensor("wq", [H, DSH], F32R, kind="ExternalInput").ap(),
        "wk": nc.dram_tensor("wk", [H, DSH], F32R, kind="ExternalInput").ap(),
        "wv": nc.dram_tensor("wv", [H, DSH], F32R, kind="ExternalInput").ap(),
        "wo": nc.dram_tensor("wo", [DSH, H], F32R, kind="ExternalInput").ap(),
        "bq": nc.dram_tensor("bq", [128, MT], F32, kind="ExternalInput").ap(),
        "bk": nc.dram_tensor("bk", [128, MT], F32, kind="ExternalInput").ap(),
        "bvb": nc.dram_tensor("bvb", [128, DSH], F32, kind="ExternalInput").ap(),
        "m01": nc.dram_tensor("m01", [128, ST], F32, kind="ExternalInput").ap(),
        "m01c": nc.dram_tensor(
            "m01c", [128, ST, NHL, 1], F32R, kind="ExternalInput"
        ).ap(),
        "outp": nc.dram_tensor("outp", [S, H], F32, kind="ExternalOutput").ap(),
    }
    with tile.TileContext(nc) as tc:
        _body(tc, nc, mybir, d)
    nc.compile()
    _cache["nc"] = nc
    return nc


def _in_maps(x, mask, wq, bq, wk, bk, wv, bv, wo, bo):
    maps = []
    for c in range(NCORES):
        b, g = divmod(c, 2)
        sl = slice(g * DSH, (g + 1) * DSH)
        m01 = (~mask[b]).astype(np.float32)  # 1.0 = keep, 0.0 = masked key
        m01_pm = np.ascontiguousarray(m01.reshape(ST, 128).T)
        maps.append(
            {
                "xt": np.ascontiguousarray(x[b].T),
                "wq": np.ascontiguousarray(wq[:, sl]),
                "wk": np.ascontiguousarray(wk[:, sl]),
                "wv": np.ascontiguousarray(wv[:, sl]),
                "wo": np.ascontiguousarray(wo[sl, :]),
                "bq": np.ascontiguousarray(bq[sl].reshape(MT, 128).T),
                "bk": np.ascontiguousarray(bk[sl].reshape(MT, 128).T),
                "bvb": np.ascontiguousarray(
                    np.broadcast_to(bv[sl][None, :], (128, DSH))
                ),
                "m01": m01_pm,
                "m01c": np.ascontiguousarray(
                    np.broadcast_to(m01_pm[:, :, None, None], (128, ST, NHL, 1))
                ),
            }
        )
    return maps


def _get_runner():
    """Build (once) a cached jitted SPMD executable over the 8 cores.

    Replicates bass2jax.run_bass_via_pjrt's multi-core path, but holds on
    to the jitted function so repeat kernel() calls don't re-lower or
    re-run the NEFF compile.
    """
    if "runner" in _cache:
        return _cache["runner"]
    import jax
    from jax.experimental.shard_map import shard_map
    from jax.sharding import Mesh, PartitionSpec
    from concourse import bass2jax, mybir

    nc = _build()
    bass2jax.install_neuronx_cc_hook()
    partition_name = (
        nc.partition_id_tensor.name if nc.partition_id_tensor else None
    )

    in_names, out_names, out_avals, zero_outs = [], [], [], []
    for alloc in nc.m.functions[0].allocations:
        if not isinstance(alloc, mybir.MemoryLocationSet):
            continue
        name = alloc.memorylocations[0].name
        if alloc.kind == "ExternalInput":
            if name != partition_name:
                in_names.append(name)
        elif alloc.kind == "ExternalOutput":
            shape = tuple(alloc.tensor_shape)
            dtype = mybir.dt.np(alloc.dtype)
            out_avals.append(jax.core.ShapedArray(shape, dtype))
            out_names.append(name)
            zero_outs.append(np.zeros(shape, dtype))
    n_params = len(in_names)
    n_outs = len(out_avals)
    all_names = in_names + out_names
    if partition_name is not None:
        all_names = all_names + [partition_name]
    donate = tuple(range(n_params, n_params + n_outs))

    def _body(*args):
        operands = list(args)
        if partition_name is not None:
            operands.append(bass2jax.partition_id_tensor())
        outs = bass2jax._bass_exec_p.bind(
            *operands,
            out_avals=tuple(out_avals),
            in_names=tuple(all_names),
            out_names=tuple(out_names),
            lowering_input_output_aliases=(),
            sim_require_finite=True,
            sim_require_nnan=True,
            nc=nc,
        )
        return tuple(outs)

    devices = jax.devices()[:NCORES]
    mesh = Mesh(np.asarray(devices), ("core",))
    sharded = jax.jit(
        shard_map(
            _body,
            mesh=mesh,
            in_specs=(PartitionSpec("core"),) * (n_params + n_outs),
            out_specs=(PartitionSpec("core"),) * n_outs,
            check_rep=False,
        ),
        donate_argnums=donate,
        keep_unused=True,
    )

    def run(maps):
        concat_in = [
            np.concatenate([np.asarray(m[name]) for m in maps], axis=0)
            for name in in_names
        ]
        concat_zeros = [
            np.zeros((NCORES * z.shape[0], *z.shape[1:]), z.dtype) for z in zero_outs
        ]
        out_arrs = sharded(*concat_in, *concat_zeros)
        return [
            {
                name: np.asarray(out_arrs[i]).reshape(NCORES, *out_avals[i].shape)[c]
                for i, name in enumerate(out_names)
            }
            for c in range(NCORES)
        ]

    _cache["runner"] = run
    return run


def kernel(**inputs):
    np_in = {k: np.asarray(v) for k, v in inputs.items()}
    x = np_in["x"].astype(np.float32, copy=False)
    mask = np_in["mask"].astype(bool, copy=False)
    args = [
        np_in[k].astype(np.float32, copy=False)
        for k in ("wq", "bq", "wk", "bk", "wv", "bv", "wo", "bo")
    ]
    wq, bq, wk, bk, wv, bv, wo, bo = args

    run = _get_runner()
    maps = _in_maps(x, mask, wq, bq, wk, bk, wv, bv, wo, bo)
    results = run(maps)
    outs = [r["outp"] for r in results]
    out = np.stack([outs[2 * b] + outs[2 * b + 1] + bo[None, :] for b in range(B)])
    return out.astype(np.float32)
